# revision 18
# baseline (speedup 1.0000x reference)
"""Bahdanau-attention kernel for TRN2 (8 NeuronCores, batch-parallel).

Computes, per batch b:
    enc_last = encoder_out[b, -1, :]                      # [1024]
    w1       = enc_last @ W1_w.T + W1_b                   # [1024]   (host)
    s        = tanh(w1 + W2_b + h @ W2_w.T)               # [L, D]
    e        = h @ s.T                                    # [L, M]
    attn     = softmax(e, axis=0)                         # column softmax
    ct       = rowsum_m(attn) * enc_last                  # [L, E]  (rank-1)
Returns (ct, attn) like the reference.

Device layout is fully transposed: h enters as hT [d, l] (f32r-rounded on
host), sT = tanh(W2T-tiles.T @ hT + w1) lands [d, m], eT = sT-tiles.T @ hT
lands [m, l] so the softmax (over l) is a free-axis reduction.  attn is
produced [m, l] per batch and returned as a transposed view.  ct is rank-1
(r outer enc_last, r = attn row-sums), so it is assembled on the host from
the returned attn — the device's job is the two big matmuls + softmax.

Matmuls run as float32r (fp32 with 11 mantissa bits): full bf16-rate on the
PE array, ~16x more accurate than bf16 — needed because std(e) ~ 16 makes
the softmax amplify any matmul error by exp().
"""

import numpy as np

B, L, D = 32, 1024, 1024
NCORES = 8
BPC = B // NCORES  # batches per core
NT = L // 128      # 128-tiles per 1024 dim
TRACE = False      # test harness may flip this for profiling

_cache = {}


def _round_f32r(x):
    """Round fp32 -> fp32r (11 mantissa bits, RNE). Matches HW cast."""
    u = np.ascontiguousarray(x).view(np.uint32)
    low = u & np.uint32(0xFFF)
    base = (u & np.uint32(0xFFFFF000)).astype(np.uint64)
    add = (
        (low > 0x800) | ((low == 0x800) & (((u >> 12) & 1) == 1))
    ).astype(np.uint64) << 12
    return ((base + add) & np.uint64(0xFFFFFFFF)).astype(np.uint32).view(np.float32)


def _build_program():
    import concourse.bass as bass  # noqa: F401
    from concourse import bacc
    import concourse.mybir as mybir
    import concourse.tile as tile

    f32 = mybir.dt.float32
    f32r = mybir.dt.float32r

    nc = bacc.Bacc(target_bir_lowering=False, debug=False, num_devices=NCORES)

    ht_ext = nc.declare_dram_parameter("ht", [BPC, NT, 128, L], f32r, isOutput=False)
    w2t_ext = nc.declare_dram_parameter("w2t", [NT, 128, D], f32r, isOutput=False)
    w1_ext = nc.declare_dram_parameter("w1", [BPC, 128, NT], f32, isOutput=False)
    attn_ext = nc.declare_dram_parameter("attn_t", [BPC, L, L], f32, isOutput=True)

    with tile.TileContext(nc) as tc:
        with (
            tc.tile_pool(name="sb", bufs=2) as sb,
            tc.tile_pool(name="ps", bufs=2, space="PSUM") as ps,
        ):
            w2t_sb = [None] * NT

            for b in range(BPC):
                # --- per-batch loads (batch-0 interleaves the weight tiles
                # --- so phase A can consume k-tiles as they stream in) ---
                ht_sb = []
                for k in range(NT):
                    t = sb.tile([128, L], f32r, tag=f"ht{k}", name=f"ht{b}_{k}", bufs=2)
                    if b == 0:
                        w = sb.tile([128, D], f32r, tag=f"w2t{k}", name=f"w2t{k}", bufs=1)
                        nc.sync.dma_start(w[:], w2t_ext[k])
                        w2t_sb[k] = w
                        # halves: the k-major c=0 sweep only waits on the
                        # first half of each k-tile; second halves trickle in
                        # two k-tiles behind so the c=1 sweep isn't starved
                        nc.sync.dma_start(t[:, 0:512], ht_ext[b, k, :, 0:512])
                        if k >= 2:
                            nc.sync.dma_start(
                                ht_sb[k - 2][:, 512:1024],
                                ht_ext[b, k - 2, :, 512:1024],
                            )
                    else:
                        nc.sync.dma_start(t[:], ht_ext[b, k])
                    ht_sb.append(t)
                if b == 0:
                    for k in range(NT - 2, NT):
                        nc.sync.dma_start(
                            ht_sb[k][:, 512:1024], ht_ext[b, k, :, 512:1024]
                        )
                w1_sb = sb.tile([128, NT], f32, tag="w1", name=f"w1_{b}", bufs=2)
                nc.sync.dma_start(w1_sb[:], w1_ext[b])

                # --- phase A: sT[d, m] = tanh(w1[d] + sum_k w2t[k,d]*ht[k, m]) ---
                st_sb = [
                    sb.tile([128, L], f32r, tag=f"st{i}", name=f"st{b}_{i}", bufs=1)
                    for i in range(NT)
                ]
                if b == 0:
                    # k-major sweeps: consume each arriving (w2t, ht) k-tile
                    # across all 8 d-tile accumulators (uses all 8 psum banks:
                    # 3 "pe" tiles (2 halves each) + 2 "pg" tiles).
                    for c in range(2):
                        acc = []
                        pe3 = [
                            ps.tile([128, L], f32, tag="pe", name=f"peA{c}_{i}", bufs=3)
                            for i in range(3)
                        ]
                        pg2 = [
                            ps.tile([128, 512], f32, tag="pg", name=f"pgA{c}_{i}", bufs=2)
                            for i in range(2)
                        ]
                        for i in range(3):
                            acc.append(pe3[i][:, 0:512])
                            acc.append(pe3[i][:, 512:1024])
                        acc.append(pg2[0][:])
                        acc.append(pg2[1][:])
                        for k in range(NT):
                            for i in range(NT):
                                nc.tensor.matmul(
                                    acc[i],
                                    w2t_sb[k][:, i * 128:(i + 1) * 128],
                                    ht_sb[k][:, c * 512:(c + 1) * 512],
                                    start=(k == 0),
                                    stop=(k == NT - 1),
                                )
                        for i in range(NT):
                            nc.scalar.activation(
                                st_sb[i][:, c * 512:(c + 1) * 512],
                                acc[i],
                                mybir.ActivationFunctionType.Tanh,
                                bias=w1_sb[:, i:i + 1],
                                scale=1.0,
                            )
                else:
                    for i in range(NT):
                        pg = [
                            ps.tile([128, 512], f32, tag="pg", name=f"pg{b}_{i}_{c}", bufs=2)
                            for c in range(2)
                        ]
                        for k in range(NT):
                            for c in range(2):
                                nc.tensor.matmul(
                                    pg[c][:],
                                    w2t_sb[k][:, i * 128:(i + 1) * 128],
                                    ht_sb[k][:, c * 512:(c + 1) * 512],
                                    start=(k == 0),
                                    stop=(k == NT - 1),
                                )
                        for c in range(2):
                            nc.scalar.activation(
                                st_sb[i][:, c * 512:(c + 1) * 512],
                                pg[c][:],
                                mybir.ActivationFunctionType.Tanh,
                                bias=w1_sb[:, i:i + 1],
                                scale=1.0,
                            )

                # --- phase B: eT[m, l] per m-tile; softmax over l ---
                for j in range(NT):
                    last_tile = b == BPC - 1 and j == NT - 1
                    pe = ps.tile([128, L], f32, tag="pe", name=f"pe{b}_{j}", bufs=3)
                    if last_tile:
                        # chunk-major: chunk 0's stats overlap chunk 1's
                        # matmuls, shortening the exposed kernel tail
                        for c in range(2):
                            for dc in range(NT):
                                nc.tensor.matmul(
                                    pe[:, c * 512:(c + 1) * 512],
                                    st_sb[dc][:, j * 128:(j + 1) * 128],
                                    ht_sb[dc][:, c * 512:(c + 1) * 512],
                                    start=(dc == 0),
                                    stop=(dc == NT - 1),
                                )
                        mxs = [
                            sb.tile([128, 1], f32, tag=f"lmx{c}", name=f"lmx{c}", bufs=1)
                            for c in range(2)
                        ]
                        for c in range(2):
                            nc.vector.reduce_max(
                                mxs[c][:],
                                pe[:, c * 512:(c + 1) * 512],
                                axis=mybir.AxisListType.X,
                            )
                        nmx = sb.tile([128, 1], f32, tag="nmx", name=f"nmx_{b}_{j}", bufs=2)
                        nc.vector.tensor_max(nmx[:], mxs[0][:], mxs[1][:])
                        nc.vector.tensor_scalar_mul(nmx[:], nmx[:], -1.0)
                        ex = sb.tile([128, L], f32, tag="ex", name=f"ex{b}_{j}", bufs=3)
                        sss = [
                            sb.tile([128, 1], f32, tag=f"lss{c}", name=f"lss{c}", bufs=1)
                            for c in range(2)
                        ]
                        for c in range(2):
                            nc.scalar.activation(
                                ex[:, c * 512:(c + 1) * 512],
                                pe[:, c * 512:(c + 1) * 512],
                                mybir.ActivationFunctionType.Exp,
                                bias=nmx[:, 0:1],
                                scale=1.0,
                                accum_out=sss[c][:],
                            )
                        tot = sb.tile([128, 1], f32, tag="tot", name=f"tot_{b}_{j}", bufs=2)
                        nc.vector.tensor_add(tot[:], sss[0][:], sss[1][:])
                        rec = sb.tile([128, 1], f32, tag="rec", name=f"rec_{b}_{j}", bufs=2)
                        nc.vector.reciprocal(rec[:], tot[:])
                        at = sb.tile([128, L], f32, tag="at", name=f"at_{b}_{j}", bufs=3)
                        for c in range(2):
                            nc.vector.tensor_scalar_mul(
                                at[:, c * 512:(c + 1) * 512],
                                ex[:, c * 512:(c + 1) * 512],
                                rec[:, 0:1],
                            )
                            nc.sync.dma_start(
                                attn_ext[
                                    b,
                                    j * 128:(j + 1) * 128,
                                    c * 512:(c + 1) * 512,
                                ],
                                at[:, c * 512:(c + 1) * 512],
                            )
                        continue
                    for dc in range(NT):
                        for c in range(2):
                            nc.tensor.matmul(
                                pe[:, c * 512:(c + 1) * 512],
                                st_sb[dc][:, j * 128:(j + 1) * 128],
                                ht_sb[dc][:, c * 512:(c + 1) * 512],
                                start=(dc == 0),
                                stop=(dc == NT - 1),
                            )
                    nmx = sb.tile([128, 1], f32, tag="nmx", name=f"nmx_{b}_{j}", bufs=2)
                    nc.vector.reduce_max(
                        nmx[:], pe[:], axis=mybir.AxisListType.X, negate=True
                    )
                    ex = sb.tile([128, L], f32, tag="ex", name=f"ex{b}_{j}", bufs=3)
                    tot = sb.tile([128, 1], f32, tag="tot", name=f"tot_{b}_{j}", bufs=2)
                    nc.scalar.activation(
                        ex[:],
                        pe[:],
                        mybir.ActivationFunctionType.Exp,
                        bias=nmx[:, 0:1],
                        scale=1.0,
                        accum_out=tot[:],
                    )
                    rec = sb.tile([128, 1], f32, tag="rec", name=f"rec_{b}_{j}", bufs=2)
                    nc.vector.reciprocal(rec[:], tot[:])
                    at = sb.tile([128, L], f32, tag="at", name=f"at_{b}_{j}", bufs=3)
                    nc.vector.tensor_scalar_mul(at[:], ex[:], rec[:, 0:1])
                    nc.sync.dma_start(attn_ext[b, j * 128:(j + 1) * 128, :], at[:])

    nc.compile()
    return nc


def _get_program():
    if "nc" not in _cache:
        _cache["nc"] = _build_program()
    return _cache["nc"]


def kernel(encoder_hid, encoder_out, mask, W1_w, W1_b, W2_w, W2_b):
    from concourse.bass_utils import run_bass_kernel_spmd

    encoder_hid = np.asarray(encoder_hid, dtype=np.float32)
    encoder_out = np.asarray(encoder_out, dtype=np.float32)
    W1_w = np.asarray(W1_w, dtype=np.float32)
    W1_b = np.asarray(W1_b, dtype=np.float32)
    W2_w = np.asarray(W2_w, dtype=np.float32)
    W2_b = np.asarray(W2_b, dtype=np.float32)

    enc_last = encoder_out[:, -1, :]                      # [B, D]
    w1_full = enc_last @ W1_w.T + W1_b + W2_b             # [B, D] (tanh bias)
    w2t = _round_f32r(W2_w.T).reshape(NT, 128, D)

    in_maps = []
    for c in range(NCORES):
        sl = slice(c * BPC, (c + 1) * BPC)
        ht = _round_f32r(
            encoder_hid[sl].transpose(0, 2, 1)
        ).reshape(BPC, NT, 128, L)
        w1c = np.ascontiguousarray(
            w1_full[sl].reshape(BPC, NT, 128).transpose(0, 2, 1)
        )
        in_maps.append({"ht": ht, "w2t": w2t, "w1": w1c})

    nc = _get_program()
    res = run_bass_kernel_spmd(nc, in_maps, list(range(NCORES)), trace=TRACE)
    if TRACE:
        _cache["exec_time_ns"] = res.exec_time_ns
        _cache["res"] = res

    attn_t = np.concatenate([r["attn_t"] for r in res.results], axis=0)  # [B, m, l]
    attn = attn_t.swapaxes(1, 2)                                         # [B, l, m]
    # ct is rank-1: ct[b] = r[b] (x) enc_last[b], r = attn row-sums
    r = attn_t.sum(axis=1)                                               # [B, l]
    ct = r[:, :, None] * enc_last[:, None, :]                            # [B, l, e]
    return ct, attn


# revision 20
# speedup vs baseline: 1.0093x; 1.0093x over previous
"""Bahdanau-attention kernel for TRN2 (8 NeuronCores, batch-parallel).

Computes, per batch b:
    enc_last = encoder_out[b, -1, :]                      # [1024]
    w1       = enc_last @ W1_w.T + W1_b                   # [1024]   (host)
    s        = tanh(w1 + W2_b + h @ W2_w.T)               # [L, D]
    e        = h @ s.T                                    # [L, M]
    attn     = softmax(e, axis=0)                         # column softmax
    ct       = rowsum_m(attn) * enc_last                  # [L, E]  (rank-1)
Returns (ct, attn) like the reference.

Device layout is fully transposed: h enters as hT [d, l] (f32r-rounded on
host), sT = tanh(W2T-tiles.T @ hT + w1) lands [d, m], eT = sT-tiles.T @ hT
lands [m, l] so the softmax (over l) is a free-axis reduction.  attn is
produced [m, l] per batch and returned as a transposed view.  ct is rank-1
(r outer enc_last, r = attn row-sums), so it is assembled on the host from
the returned attn — the device's job is the two big matmuls + softmax.

Matmuls run as float32r (fp32 with 11 mantissa bits): full bf16-rate on the
PE array, ~16x more accurate than bf16 — needed because std(e) ~ 16 makes
the softmax amplify any matmul error by exp().
"""

import numpy as np

B, L, D = 32, 1024, 1024
NCORES = 8
BPC = B // NCORES  # batches per core
NT = L // 128      # 128-tiles per 1024 dim
TRACE = False      # test harness may flip this for profiling

_cache = {}


def _round_f32r(x):
    """Round fp32 -> fp32r (11 mantissa bits, RNE). Matches HW cast."""
    u = np.ascontiguousarray(x).view(np.uint32)
    low = u & np.uint32(0xFFF)
    base = (u & np.uint32(0xFFFFF000)).astype(np.uint64)
    add = (
        (low > 0x800) | ((low == 0x800) & (((u >> 12) & 1) == 1))
    ).astype(np.uint64) << 12
    return ((base + add) & np.uint64(0xFFFFFFFF)).astype(np.uint32).view(np.float32)


def _build_program():
    import concourse.bass as bass  # noqa: F401
    from concourse import bacc
    import concourse.mybir as mybir
    import concourse.tile as tile

    f32 = mybir.dt.float32
    f32r = mybir.dt.float32r

    nc = bacc.Bacc(target_bir_lowering=False, debug=False, num_devices=NCORES)

    ht_ext = nc.declare_dram_parameter("ht", [BPC, NT, 128, L], f32r, isOutput=False)
    w2t_ext = nc.declare_dram_parameter("w2t", [NT, 128, D], f32r, isOutput=False)
    w1_ext = nc.declare_dram_parameter("w1", [BPC, 128, NT], f32, isOutput=False)
    attn_ext = nc.declare_dram_parameter("attn_t", [BPC, L, L], f32, isOutput=True)

    with tile.TileContext(nc) as tc:
        with (
            tc.tile_pool(name="sb", bufs=2) as sb,
            tc.tile_pool(name="ps", bufs=2, space="PSUM") as ps,
        ):
            w2t_sb = [None] * NT

            for b in range(BPC):
                # --- per-batch loads (batch-0 interleaves the weight tiles
                # --- so phase A can consume k-tiles as they stream in) ---
                ht_sb = []
                for k in range(NT):
                    t = sb.tile([128, L], f32r, tag=f"ht{k}", name=f"ht{b}_{k}", bufs=2)
                    if b == 0:
                        w = sb.tile([128, D], f32r, tag=f"w2t{k}", name=f"w2t{k}", bufs=1)
                        nc.sync.dma_start(w[:], w2t_ext[k])
                        w2t_sb[k] = w
                        # halves: the k-major c=0 sweep only waits on the
                        # first half of each k-tile; second halves trickle in
                        # two k-tiles behind so the c=1 sweep isn't starved
                        nc.sync.dma_start(t[:, 0:512], ht_ext[b, k, :, 0:512])
                        if k >= 2:
                            nc.sync.dma_start(
                                ht_sb[k - 2][:, 512:1024],
                                ht_ext[b, k - 2, :, 512:1024],
                            )
                    else:
                        nc.sync.dma_start(t[:], ht_ext[b, k])
                    ht_sb.append(t)
                if b == 0:
                    for k in range(NT - 2, NT):
                        nc.sync.dma_start(
                            ht_sb[k][:, 512:1024], ht_ext[b, k, :, 512:1024]
                        )
                w1_sb = sb.tile([128, NT], f32, tag="w1", name=f"w1_{b}", bufs=2)
                nc.sync.dma_start(w1_sb[:], w1_ext[b])

                # --- phase A: sT[d, m] = tanh(w1[d] + sum_k w2t[k,d]*ht[k, m]) ---
                st_sb = [
                    sb.tile([128, L], f32r, tag=f"st{i}", name=f"st{b}_{i}", bufs=1)
                    for i in range(NT)
                ]
                if b == 0:
                    # k-major sweeps: consume each arriving (w2t, ht) k-tile
                    # across all 8 d-tile accumulators (uses all 8 psum banks:
                    # 3 "pe" tiles (2 halves each) + 2 "pg" tiles).
                    for c in range(2):
                        acc = []
                        pe3 = [
                            ps.tile([128, L], f32, tag="pe", name=f"peA{c}_{i}", bufs=3)
                            for i in range(3)
                        ]
                        pg2 = [
                            ps.tile([128, 512], f32, tag="pg", name=f"pgA{c}_{i}", bufs=2)
                            for i in range(2)
                        ]
                        for i in range(3):
                            acc.append(pe3[i][:, 0:512])
                            acc.append(pe3[i][:, 512:1024])
                        acc.append(pg2[0][:])
                        acc.append(pg2[1][:])
                        for k in range(NT):
                            for i in range(NT):
                                nc.tensor.matmul(
                                    acc[i],
                                    w2t_sb[k][:, i * 128:(i + 1) * 128],
                                    ht_sb[k][:, c * 512:(c + 1) * 512],
                                    start=(k == 0),
                                    stop=(k == NT - 1),
                                )
                        for i in range(NT):
                            nc.scalar.activation(
                                st_sb[i][:, c * 512:(c + 1) * 512],
                                acc[i],
                                mybir.ActivationFunctionType.Tanh,
                                bias=w1_sb[:, i:i + 1],
                                scale=1.0,
                            )
                else:
                    for i in range(NT):
                        pg = [
                            ps.tile([128, 512], f32, tag="pg", name=f"pg{b}_{i}_{c}", bufs=2)
                            for c in range(2)
                        ]
                        for k in range(NT):
                            for c in range(2):
                                nc.tensor.matmul(
                                    pg[c][:],
                                    w2t_sb[k][:, i * 128:(i + 1) * 128],
                                    ht_sb[k][:, c * 512:(c + 1) * 512],
                                    start=(k == 0),
                                    stop=(k == NT - 1),
                                )
                        for c in range(2):
                            nc.scalar.activation(
                                st_sb[i][:, c * 512:(c + 1) * 512],
                                pg[c][:],
                                mybir.ActivationFunctionType.Tanh,
                                bias=w1_sb[:, i:i + 1],
                                scale=1.0,
                            )

                # --- phase B: eT[m, l] per m-tile; softmax over l ---
                for j in range(NT):
                    pe = ps.tile([128, L], f32, tag="pe", name=f"pe{b}_{j}", bufs=3)
                    for dc in range(NT):
                        for c in range(2):
                            nc.tensor.matmul(
                                pe[:, c * 512:(c + 1) * 512],
                                st_sb[dc][:, j * 128:(j + 1) * 128],
                                ht_sb[dc][:, c * 512:(c + 1) * 512],
                                start=(dc == 0),
                                stop=(dc == NT - 1),
                            )
                    nmx = sb.tile([128, 1], f32, tag="nmx", name=f"nmx_{b}_{j}", bufs=2)
                    nc.vector.reduce_max(
                        nmx[:], pe[:], axis=mybir.AxisListType.X, negate=True
                    )
                    ex = sb.tile([128, L], f32, tag="ex", name=f"ex{b}_{j}", bufs=3)
                    tot = sb.tile([128, 1], f32, tag="tot", name=f"tot_{b}_{j}", bufs=2)
                    nc.scalar.activation(
                        ex[:],
                        pe[:],
                        mybir.ActivationFunctionType.Exp,
                        bias=nmx[:, 0:1],
                        scale=1.0,
                        accum_out=tot[:],
                    )
                    rec = sb.tile([128, 1], f32, tag="rec", name=f"rec_{b}_{j}", bufs=2)
                    nc.vector.reciprocal(rec[:], tot[:])
                    at = sb.tile([128, L], f32, tag="at", name=f"at_{b}_{j}", bufs=3)
                    nc.vector.tensor_scalar_mul(at[:], ex[:], rec[:, 0:1])
                    nc.sync.dma_start(attn_ext[b, j * 128:(j + 1) * 128, :], at[:])

    nc.compile()
    return nc


def _get_program():
    if "nc" not in _cache:
        _cache["nc"] = _build_program()
    return _cache["nc"]


def kernel(encoder_hid, encoder_out, mask, W1_w, W1_b, W2_w, W2_b):
    from concourse.bass_utils import run_bass_kernel_spmd

    encoder_hid = np.asarray(encoder_hid, dtype=np.float32)
    encoder_out = np.asarray(encoder_out, dtype=np.float32)
    W1_w = np.asarray(W1_w, dtype=np.float32)
    W1_b = np.asarray(W1_b, dtype=np.float32)
    W2_w = np.asarray(W2_w, dtype=np.float32)
    W2_b = np.asarray(W2_b, dtype=np.float32)

    enc_last = encoder_out[:, -1, :]                      # [B, D]
    w1_full = enc_last @ W1_w.T + W1_b + W2_b             # [B, D] (tanh bias)
    w2t = _round_f32r(W2_w.T).reshape(NT, 128, D)

    in_maps = []
    for c in range(NCORES):
        sl = slice(c * BPC, (c + 1) * BPC)
        ht = _round_f32r(
            encoder_hid[sl].transpose(0, 2, 1)
        ).reshape(BPC, NT, 128, L)
        w1c = np.ascontiguousarray(
            w1_full[sl].reshape(BPC, NT, 128).transpose(0, 2, 1)
        )
        in_maps.append({"ht": ht, "w2t": w2t, "w1": w1c})

    nc = _get_program()
    res = run_bass_kernel_spmd(nc, in_maps, list(range(NCORES)), trace=TRACE)
    if TRACE:
        _cache["exec_time_ns"] = res.exec_time_ns
        _cache["res"] = res

    attn_t = np.concatenate([r["attn_t"] for r in res.results], axis=0)  # [B, m, l]
    attn = attn_t.swapaxes(1, 2)                                         # [B, l, m]
    # ct is rank-1: ct[b] = r[b] (x) enc_last[b], r = attn row-sums
    r = attn_t.sum(axis=1)                                               # [B, l]
    ct = r[:, :, None] * enc_last[:, None, :]                            # [B, l, e]
    return ct, attn
